# revision 1
# baseline (speedup 1.0000x reference)
"""Trainium2 Bass kernel for nn_LinkEncoding (gnn_message_passing).

Takes FULL inputs (as produced by reference.setup_inputs()), shards
data-parallel over nodes across 8 NeuronCores, runs a Bass/Tile kernel,
returns the FULL [N, OUT_CH] float32 output.

v2 design notes (vs the v1 baseline, 8.77ms -> 3.41ms):
  - Scalar/ACT engine runs ONLY Gelu/Identity/Copy (all live in the same
    activation table) -> zero ACT_TABLE_LOADs (v1 burned 2.4ms on them).
    Sqrt is gone: 1/sqrt(var+eps) is computed on DVE with the quake
    exponent-halving trick done in *float* arithmetic on the int bit
    pattern (i32-value reads/writes; HW rejects int scalar operands)
    plus one Newton step.
  - LN normalize no longer uses 4 per-t Identity activations.  A single
    wide DVE tensor_tensor multiplies the PSUM tile by a per-(slot,t)
    broadcast rsqrt while making the bf16 SBUF copy the next matmul needs
    anyway.  For LN_t the -mean*rsqrt is pre-added (2-byte DVE add); for
    LN_c/LN_h it rides an augmented column through the transpose into
    ch1's w1-rowsum row / the out-projection's owt-rowsum row.
  - tok1/tok2 are single merged matmuls over all 4 sub-tiles (constant
    stationary operands); mean-over-K + LN_h fold into a tiny per-group
    ri-weighted matmul (R pattern) accumulated into an SBUF strip.
  - the emission loop is software-pipelined ~4 groups deep over 9 fine
    blocks (P2b,P1,V3b,P4,V2,P3b,P3a,V1,V3a,P2a) so each in-order
    engine queue sees work whose deps were emitted >= 1 iteration ago;
    PSUM banks: Bx x4, Btp/TM16 shared, zcT/Bc/P2f shared, Bh x2.
  - output written channel-major to DRAM, transposed on host.
"""

import math
import os
import sys

for _p in ("/opt/trn_rl_repo", "/root/.axon_site/_ro/trn_rl_repo"):
    if os.path.isdir(_p) and _p not in sys.path:
        sys.path.append(_p)

import numpy as np
import ml_dtypes

BF16 = ml_dtypes.bfloat16

# Problem constants (hardcoded per harness contract).
N_NODES = 50000
E_EDGES = 800000
K = 30
HID = 100
TCH = 100
OUT_CH = 100
NCORES = 8

SLOT = 4 * K          # 120 slots (4 nodes) per sub-tile
GW = 16               # nodes per group (4 sub-tiles)
CH_G = 32             # groups per chunk (TM [101, 512] f32 = 1 bank)

MAGIC = 0x5F3759DF

_CACHE = {}
LAST_RESULT = None


def _bcast(ap, n):
    """Broadcast the last (size-1) free dim of `ap` to size n via stride 0."""
    import concourse.bass as bass
    a = list(ap.ap)
    assert a[-1][1] == 1, a
    a[-1] = [0, n]
    return bass.AP(tensor=ap.tensor, offset=ap.offset, ap=a)


def _bcast_mid(ap, n):
    """Insert a stride-0 dim of size n after the partition dim."""
    import concourse.bass as bass
    a = list(ap.ap)
    a.insert(1, [0, n])
    return bass.AP(tensor=ap.tensor, offset=ap.offset, ap=a)


DEBUG_TAPS = False


def _build_nc(NG, lnt_identity, newton_iters=1):
    import concourse.bass as bass
    import concourse.tile as tile
    from concourse import bacc, mybir
    from contextlib import ExitStack

    f32 = mybir.dt.float32
    bf16 = mybir.dt.bfloat16
    i32 = mybir.dt.int32
    AF = mybir.ActivationFunctionType
    OP = mybir.AluOpType

    nc = bacc.Bacc(None, target_bir_lowering=False)

    xin = nc.dram_tensor("xin", [NG, 106, 4 * SLOT], bf16, kind="ExternalInput")
    wht = nc.dram_tensor("wht", [106, HID], bf16, kind="ExternalInput")
    t1 = nc.dram_tensor("t1", [SLOT, 64], bf16, kind="ExternalInput")
    t2 = nc.dram_tensor("t2", [64, SLOT], bf16, kind="ExternalInput")
    w1 = nc.dram_tensor("w1", [101, 4, HID], bf16, kind="ExternalInput")
    w2 = nc.dram_tensor("w2", [HID, 4, HID], bf16, kind="ExternalInput")
    owt = nc.dram_tensor("owt", [101, HID], f32, kind="ExternalInput")
    idb = nc.dram_tensor("idb", [SLOT, SLOT], bf16, kind="ExternalInput")
    idf = nc.dram_tensor("idf", [HID, HID], f32, kind="ExternalInput")
    b1 = nc.dram_tensor("b1", [HID, 4], f32, kind="ExternalInput")
    b2 = nc.dram_tensor("b2", [HID, 1], f32, kind="ExternalInput")
    ob = nc.dram_tensor("ob", [HID, 1], f32, kind="ExternalInput")
    t1b = nc.dram_tensor("t1b", [64, 1], f32, kind="ExternalInput")
    pk = nc.dram_tensor("pk", [SLOT, 4], f32, kind="ExternalInput")
    gtb = nc.dram_tensor("gtb", [SLOT, 2 * HID], f32, kind="ExternalInput")
    y2 = nc.dram_tensor("y2", [HID, NG * GW], f32, kind="ExternalOutput")
    if DEBUG_TAPS:
        dbgs = {}
        for nmx, shp in [("head", [NG, SLOT, 400]), ("xs", [NG, SLOT, 400]),
                         ("btp", [NG, 64, 400]), ("htok", [NG, SLOT, 400]),
                         ("zcs", [NG, SLOT, 404]), ("zst", [NG, 101, 480]),
                         ("hcs", [NG, HID, 480]), ("hch", [NG, SLOT, 400]),
                         ("hhs", [NG, SLOT, 404]), ("rr", [NG, SLOT, 16]),
                         ("tm", [NG, 101, 16])]:
            dbgs[nmx] = nc.dram_tensor("dbg_" + nmx, shp, f32,
                                       kind="ExternalOutput")

    with tile.TileContext(nc) as tc, ExitStack() as ctx:
        singles = ctx.enter_context(tc.tile_pool(name="singles", bufs=1))
        pgt = ctx.enter_context(tc.tile_pool(name="pgt", bufs=6))
        px = ctx.enter_context(tc.tile_pool(name="px", bufs=5))
        pgel = ctx.enter_context(tc.tile_pool(name="pgel", bufs=4))
        pzt = ctx.enter_context(tc.tile_pool(name="pzt", bufs=4))
        pgh = ctx.enter_context(tc.tile_pool(name="pgh", bufs=4))
        phc = ctx.enter_context(tc.tile_pool(name="phc", bufs=4))
        pstat = ctx.enter_context(tc.tile_pool(name="pstat", bufs=5))
        ptm = ctx.enter_context(tc.tile_pool(name="ptm", bufs=2))
        # PSUM pools: 2 + 2 + 2 + 2 = 8 banks
        pbx = ctx.enter_context(tc.tile_pool(name="pbx", bufs=4, space="PSUM"))
        pps = ctx.enter_context(tc.tile_pool(name="pps", bufs=2, space="PSUM"))
        pbh = ctx.enter_context(tc.tile_pool(name="pbh", bufs=2, space="PSUM"))

        # --- constants -------------------------------------------------
        s_wht = singles.tile([106, HID], bf16)
        nc.sync.dma_start(s_wht[:], wht[:, :])
        s_t1 = singles.tile([SLOT, 64], bf16)
        nc.sync.dma_start(s_t1[:], t1[:, :])
        s_t2 = singles.tile([64, SLOT], bf16)
        nc.sync.dma_start(s_t2[:], t2[:, :])
        s_w1 = singles.tile([101, 4, HID], bf16)
        nc.sync.dma_start(s_w1[:], w1[:, :, :])
        s_w2 = singles.tile([HID, 4, HID], bf16)
        nc.sync.dma_start(s_w2[:], w2[:, :, :])
        s_owt = singles.tile([101, HID], f32)
        nc.sync.dma_start(s_owt[:], owt[:, :])
        s_idb = singles.tile([SLOT, SLOT], bf16)
        nc.sync.dma_start(s_idb[:], idb[:, :])
        s_idf = singles.tile([HID, HID], f32)
        nc.sync.dma_start(s_idf[:], idf[:, :])
        s_b1 = singles.tile([HID, 4], f32)
        nc.sync.dma_start(s_b1[:], b1[:, :])
        s_b2 = singles.tile([HID, 1], f32)
        nc.sync.dma_start(s_b2[:], b2[:, :])
        s_ob = singles.tile([HID, 1], f32)
        nc.sync.dma_start(s_ob[:], ob[:, :])
        s_t1b = singles.tile([64, 1], f32)
        nc.sync.dma_start(s_t1b[:], t1b[:, :])
        s_pk = singles.tile([SLOT, 4], f32)
        nc.sync.dma_start(s_pk[:], pk[:, :])
        s_gtb = singles.tile([SLOT, 2 * HID], f32)
        nc.sync.dma_start(s_gtb[:], gtb[:, :])
        s_zero = singles.tile([SLOT, 4], bf16)
        nc.vector.memset(s_zero[:], 0.0)

        if DEBUG_TAPS:
            pdbg = ctx.enter_context(tc.tile_pool(name="pdbg", bufs=2))

            def tap(nmx, g, src, P, F):
                t_ = pdbg.tile([P, F], f32, tag="dbg" + nmx)
                dst = t_[:]
                if len(src.shape) == 3:
                    dst = dst.rearrange("p (a b) -> p a b", a=src.shape[1])
                nc.vector.tensor_copy(dst, src)
                nc.sync.dma_start(dbgs[nmx][g, :, :], t_[:])
        else:
            def tap(nmx, g, src, P, F):
                pass

        def emit_stats(src3, tag):
            """src3: [SLOT, 4, HID] view of a PSUM tile.  Returns mv [SLOT,4,2]
            (mean, var) via grouped bn_stats + per-t bn_aggr.

            The HW BNStats instruction requires a 6-elem/partition output, so
            this is 4 per-t calls (grouped output is sim-only)."""
            st = pstat.tile([SLOT, 4, 6], f32, tag=tag + "st")
            for t in range(4):
                nc.vector.bn_stats(st[:, t, :], src3[:, t, :])
            mv = pstat.tile([SLOT, 4, 2], f32, tag=tag + "mv")
            for t in range(4):
                nc.vector.bn_aggr(mv[:, t, :], st[:, t, :])
            return mv

        def emit_rsqrt(mv, tag):
            """ri = 1/sqrt(mv[:,:,1] + eps) on GpSimd (bit trick + Newton).

            The exponent-halving init is done in float arithmetic on the int
            VALUE (i32<->f32 converting copies): y0_bits ~= MAGIC - bits/2.
            The ~64-ulp f32 rounding is absorbed by the Newton steps.
            """
            g = nc.vector
            ve = pstat.tile([SLOT, 4], f32, tag=tag + "ve")
            g.tensor_scalar(ve[:], mv[:, :, 1], 1e-5, None, op0=OP.add)
            # fused exponent-halving: read bits as int values, affine in
            # fp, write back rounded to int (the y0 bit pattern)
            y = pstat.tile([SLOT, 4], f32, tag=tag + "y0")
            g.tensor_scalar(y[:].bitcast(i32), ve[:].bitcast(i32), -0.5,
                            float(MAGIC), op0=OP.mult, op1=OP.add)
            for it in range(newton_iters):
                t1_ = pstat.tile([SLOT, 4], f32, tag=tag + f"nt{it}a")
                g.tensor_tensor(t1_[:], y[:], y[:], op=OP.mult)
                t2_ = pstat.tile([SLOT, 4], f32, tag=tag + f"nt{it}b")
                g.scalar_tensor_tensor(t2_[:], t1_[:], -0.5, ve[:],
                                       op0=OP.mult, op1=OP.mult)
                yn = pstat.tile([SLOT, 4], f32, tag=tag + f"nt{it}d")
                g.scalar_tensor_tensor(yn[:], t2_[:], 1.5, y[:],
                                       op0=OP.add, op1=OP.mult)
                y = yn
            return y

        nchunks = (NG + CH_G - 1) // CH_G
        PF = 3                     # DMA prefetch depth (groups)
        gts = {}                   # g -> GTs tile
        state = {}                 # g -> per-group tiles live across stages
        accs = {}                  # ci -> SBUF TM accumulator

        def issue_dma(g):
            GTs = pgt.tile([106, 4 * SLOT], bf16, tag="gt", name=f"gts{g}")
            nc.sync.dma_start(GTs[:], xin[g, :, :])
            gts[g] = GTs

        def P1(g):
            GTs = gts.pop(g)
            Bx = pbx.tile([SLOT, 4, HID], f32, tag="bx", name=f"bx{g}")
            for t in range(4):
                nc.tensor.matmul(Bx[:, t, :],
                                 GTs[:, t * SLOT:(t + 1) * SLOT],
                                 s_wht[:], start=(t == 0), stop=True,
                                 skip_group_check=True)
            state[g] = {"Bx": Bx}
            tap("head", g, Bx[:, :, :], SLOT, 400)

        def V1(g):
            Bx = state[g]["Bx"]
            mv = emit_stats(Bx[:, :, :], "t")
            ri = emit_rsqrt(mv, "t")
            xS = px.tile([SLOT, 4, HID], bf16, tag="xs")
            nc.vector.tensor_tensor(xS[:, :, :], Bx[:, :, :],
                                    _bcast(ri[:, :, None], 100), op=OP.mult)
            nmb = pstat.tile([SLOT, 4], bf16, tag="tnmb")
            nc.vector.scalar_tensor_tensor(nmb[:], mv[:, :, 0], -1.0, ri[:],
                                           op0=OP.mult, op1=OP.mult)
            nc.vector.tensor_tensor(xS[:, :, :], xS[:, :, :],
                                    _bcast(nmb[:, :, None], 100), op=OP.add)
            if lnt_identity:
                pass
            else:
                gt_ = _bcast_mid(s_gtb[:, 0:100], 4)
                bt_ = _bcast_mid(s_gtb[:, 100:200], 4)
                tmp = px.tile([SLOT, 4, HID], f32, tag="lngt")
                nc.vector.scalar_tensor_tensor(
                    tmp[:], xS[:, :, :], 1.0, gt_, op0=OP.mult,
                    op1=OP.mult)
                nc.vector.tensor_tensor(xS[:, :, :], tmp[:], bt_, op=OP.add)
            state[g]["xS"] = xS
            tap("xs", g, xS[:, :, :], SLOT, 400)

        def P2a(g):
            xS = state[g].pop("xS")
            # tok1 (merged over t, z fully normalized in xS)
            Btp = pps.tile([64, 4, HID], f32, tag="btm", bufs=1)
            nc.tensor.matmul(Btp[:, :, :], s_t1[:], xS[:, :, :],
                             start=True, stop=True, skip_group_check=True)
            tap("btp", g, Btp[:, :, :], 64, 400)
            state[g]["Btp"] = Btp

        def P2a2(g):
            Btp = state[g].pop("Btp")
            gel = pgel.tile([64, 4, HID], bf16, tag="gel")
            nc.scalar.activation(gel[:], Btp[:, :, :], AF.Gelu,
                                 bias=s_t1b[:, 0:1], scale=1.0)
            state[g]["gel"] = gel

        def P2b(g):
            Bx = state[g]["Bx"]
            gel = state[g].pop("gel")
            # tok2 (merged): h_token = x + t2.T @ gel   (accumulate)
            nc.tensor.matmul(Bx[:, :, :], s_t2[:], gel[:, :, :],
                             start=False, stop=True, skip_group_check=True)
            tap("htok", g, Bx[:, :, :], SLOT, 400)

        def V2(g):
            Bx = state[g]["Bx"]
            mvc = emit_stats(Bx[:, :, :], "c")
            ric = emit_rsqrt(mvc, "c")
            zcS = px.tile([SLOT, 4, 102], bf16, tag="zc")
            nc.vector.tensor_tensor(zcS[:, :, 0:100], Bx[:, :, :],
                                    _bcast(ric[:, :, None], 100), op=OP.mult)
            nc.vector.scalar_tensor_tensor(zcS[:, :, 100], mvc[:, :, 0],
                                           -1.0, ric[:], op0=OP.mult,
                                           op1=OP.mult)
            state[g]["zcS"] = zcS
            tap("zcs", g, zcS[:, :, 0:101], SLOT, 404)

        def P3a(g):
            zcS = state[g].pop("zcS")
            # transpose: zcT [101, (4,120)] bf16 PSUM
            zcT = pps.tile([101, 4, SLOT], bf16, tag="zcb", bufs=1)
            for t in range(4):
                nc.tensor.matmul(zcT[:, t, :], zcS[:, t, 0:101], s_idb[:],
                                 is_transpose=True, start=(t == 0),
                                 stop=True, skip_group_check=True)
            zs = pzt.tile([101, 4, SLOT], bf16, tag="zs")
            nc.scalar.activation(zs[:], zcT[:], AF.Copy)
            tap("zst", g, zs[:, :, :], 101, 480)
            state[g]["zs"] = zs

        def P3b(g):
            Bx = state[g]["Bx"]
            zs = state[g].pop("zs")
            gh = pgh.tile([HID, 4, 4 * SLOT], bf16, tag="gh")
            for j in range(4):
                Bh = pbh.tile([HID, 4 * SLOT], f32, tag="bh")
                nc.tensor.matmul(Bh[:], s_w1[:, j, :], zs[:, :, :],
                                 start=True, stop=True)
                nc.scalar.activation(gh[:, j, :], Bh[:], AF.Gelu,
                                     bias=s_b1[:, j:j + 1], scale=1.0)
            Bc = pps.tile([HID, 4 * SLOT], f32, tag="zcb", bufs=1)
            for j in range(4):
                nc.tensor.matmul(Bc[:], s_w2[:, j, :], gh[:, j, :],
                                 start=(j == 0), stop=(j == 3))
            hcS = phc.tile([HID, 4 * SLOT], f32, tag="hc")
            nc.vector.tensor_scalar(hcS[:], Bc[:], s_b2[:, 0:1], None,
                                    op0=OP.add)
            tap("hcs", g, hcS[:, :], HID, 480)
            # back-transpose + residual accumulate into Bx
            for t in range(4):
                nc.tensor.matmul(Bx[:, t, :],
                                 hcS[:, t * SLOT:(t + 1) * SLOT],
                                 s_idf[:], is_transpose=True,
                                 start=False, stop=True,
                                 skip_group_check=True)
            tap("hch", g, Bx[:, :, :], SLOT, 400)

        def V3a(g):
            Bx = state[g].pop("Bx")
            hhS = px.tile([SLOT, 4, 102], bf16, tag="hh")
            nc.scalar.activation(hhS[:, :, 0:100], Bx[:, :, :], AF.Copy)
            state[g]["hhS"] = hhS

        def V3b(g):
            hhS = state[g]["hhS"]
            mvh = emit_stats(hhS[:, :, 0:100], "h")
            rih = emit_rsqrt(mvh, "h")
            nc.vector.tensor_scalar(hhS[:, :, 100], mvh[:, :, 0], -1.0,
                                    None, op0=OP.mult)
            R = pstat.tile([SLOT, 4, 4], bf16, tag="R")
            nc.vector.tensor_tensor(R[:], _bcast_mid(s_pk[:, :], 4),
                                    _bcast(rih[:, :, None], 4), op=OP.mult)
            state[g]["R"] = R
            tap("hhs", g, hhS[:, :, 0:101], SLOT, 404)
            tap("rr", g, R[:, :, :], SLOT, 16)

        def P4(g):
            st = state.pop(g)
            hhS, R = st["hhS"], st["R"]
            assert "Bx" not in st
            ci, gi = g // CH_G, g % CH_G
            if gi == 0:
                accs[ci] = ptm.tile([101, GW * CH_G], f32, tag="acc",
                                    name=f"acc{ci}")
            acc = accs[ci]
            TM16 = pps.tile([101, 16], f32, tag="btm", bufs=1)
            for t in range(4):
                nc.tensor.matmul(TM16[:, 4 * t:4 * t + 4],
                                 hhS[:, t, 0:101], R[:, t, :],
                                 start=(t == 0), stop=True,
                                 skip_group_check=True)
            nc.vector.tensor_copy(acc[:, 16 * gi:16 * gi + 16], TM16[:])
            tap("tm", g, TM16[:, :], 101, 16)
            if gi == CH_G - 1 or g == NG - 1:
                finale(ci)

        def finale(ci):
            acc = accs.pop(ci)
            g0 = ci * CH_G
            gn = min(CH_G, NG - g0)
            nn = GW * gn
            P2f = pps.tile([HID, GW * CH_G], f32, tag="zcb", bufs=1)
            nc.tensor.matmul(P2f[:, :nn], s_owt[:], acc[:, :nn],
                             start=True, stop=True)
            pj = ptm.tile([HID, GW * CH_G], f32, tag="pj")
            nc.scalar.activation(pj[:, :nn], P2f[:, :nn], AF.Identity,
                                 bias=s_ob[:, 0:1], scale=1.0)
            nc.sync.dma_start(y2[:, g0 * GW:g0 * GW + nn], pj[:, :nn])

        # --- software-pipelined emission over 7 fine-grained blocks ---
        for g in range(min(PF, NG)):
            issue_dma(g)
        for i in range(NG + 4):
            if 0 <= i - 2 < NG:
                P2b(i - 2)
            if i < NG:
                P1(i)
                if i + PF < NG:
                    issue_dma(i + PF)
            if 0 <= i - 4 < NG:
                V3b(i - 4)
                P4(i - 4)
            if 0 <= i - 2 < NG:
                V2(i - 2)
            if 0 <= i - 3 < NG:
                P3b(i - 3)
            if 0 <= i - 2 < NG:
                P3a(i - 2)
            if 0 <= i - 1 < NG:
                V1(i - 1)
            if 0 <= i - 3 < NG:
                V3a(i - 3)
            if 0 <= i - 1 < NG:
                P2a(i - 1)
                P2a2(i - 1)
    nc.compile()
    return nc


def _host_prepare(inputs):
    """Build per-core device input maps from the full problem inputs."""
    ea = np.asarray(inputs["edge_attr"], dtype=np.float32)
    et = np.asarray(inputs["edge_time"], dtype=np.float32)
    nb = np.asarray(inputs["node_batch"]).astype(np.int64)
    N = int(np.asarray(inputs["num_nodes"]))
    E = nb.shape[0]

    head_w = np.asarray(inputs["head_w"], dtype=np.float64)
    head_b = np.asarray(inputs["head_b"], dtype=np.float64)
    ln_t_g = np.asarray(inputs["ln_t_g"], dtype=np.float64)
    ln_t_b = np.asarray(inputs["ln_t_b"], dtype=np.float64)
    tok1_w = np.asarray(inputs["tok1_w"], dtype=np.float64)
    tok1_b = np.asarray(inputs["tok1_b"], dtype=np.float64)
    tok2_w = np.asarray(inputs["tok2_w"], dtype=np.float64)
    tok2_b = np.asarray(inputs["tok2_b"], dtype=np.float64)
    ln_c_g = np.asarray(inputs["ln_c_g"], dtype=np.float64)
    ln_c_b = np.asarray(inputs["ln_c_b"], dtype=np.float64)
    ch1_w = np.asarray(inputs["ch1_w"], dtype=np.float64)
    ch1_b = np.asarray(inputs["ch1_b"], dtype=np.float64)
    ch2_w = np.asarray(inputs["ch2_w"], dtype=np.float64)
    ch2_b = np.asarray(inputs["ch2_b"], dtype=np.float64)
    ln_h_g = np.asarray(inputs["ln_h_g"], dtype=np.float64)
    ln_h_b = np.asarray(inputs["ln_h_b"], dtype=np.float64)
    out_w = np.asarray(inputs["out_w"], dtype=np.float64)
    out_b = np.asarray(inputs["out_b"], dtype=np.float64)

    NPC = (N + NCORES - 1) // NCORES          # nodes per core
    NPCP = ((NPC + GW - 1) // GW) * GW        # padded to group multiple
    NG = NPCP // GW

    # --- edge -> slot assignment (stable sort, first K per node) ---
    order = np.argsort(nb, kind="stable")
    snb = nb[order]
    pos = np.arange(E, dtype=np.int64) - np.searchsorted(snb, snb, side="left")
    keep = pos < K
    le = order[keep]                 # edge ids, slot-ordered
    lnode = snb[keep]
    lk = pos[keep]
    core = (lnode // NPC).astype(np.int64)
    nl = (lnode % NPC).astype(np.int64)

    # --- dense slot table [cores, NPCP, K, 106] bf16 ---
    dense = np.zeros((NCORES, NPCP, K, 106), dtype=BF16)
    t64 = et[le].astype(np.float64)
    t2 = t64 * t64
    tp = np.stack([t2, t2 ** 2, t2 ** 3, t2 ** 4, t2 ** 5], axis=1)
    dense[core, nl, lk, 0:5] = tp.astype(np.float32)
    dense[core, nl, lk, 5:105] = ea[le]
    dense[core, nl, lk, 105] = np.float32(1.0)

    # --- folded weights ---
    sqrt_d = math.sqrt(TCH)
    tw = 1.0 / sqrt_d ** np.linspace(0.0, sqrt_d, TCH)  # float64
    W_time = head_w[:, :TCH]
    W_attr = head_w[:, TCH:]
    C = []
    for m in range(6):
        coef = ((-1.0) ** m) / math.factorial(2 * m)
        C.append(coef * (W_time @ (tw ** (2 * m))))     # [HID]
    wht = np.zeros((106, HID), dtype=np.float32)
    for m in range(1, 6):
        wht[m - 1, :] = C[m]
    wht[5:105, :] = W_attr.T
    wht[105, :] = head_b + C[0]

    lnt_identity = bool(np.allclose(ln_t_g, 1.0) and np.allclose(ln_t_b, 0.0))

    t1m = np.zeros((SLOT, 64), dtype=np.float32)
    t2m = np.zeros((64, SLOT), dtype=np.float32)
    for b in range(4):
        t1m[30 * b:30 * b + 30, 16 * b:16 * b + 15] = tok1_w.T
        t2m[16 * b:16 * b + 15, 30 * b:30 * b + 30] = tok2_w.T
    t1bv = np.zeros((64, 1), dtype=np.float32)
    for b in range(4):
        t1bv[16 * b:16 * b + 15, 0] = tok1_b
    # tok2_b dropped: constant per-slot shift is invariant under LN_c /
    # LN_h (which are the only consumers of h_token / h_channel).

    Wg1 = ch1_w * ln_c_g[None, :]
    b1p = ch1_b + ch1_w @ ln_c_b
    w1m = np.zeros((101, 4, HID), dtype=np.float32)
    for j in range(4):
        blk = Wg1[HID * j:HID * (j + 1), :].T          # [100c, 100h]
        w1m[0:100, j, :] = blk
        w1m[100, j, :] = blk.sum(axis=0)               # w1 row-sums
    b1m = np.stack([b1p[HID * j:HID * (j + 1)] for j in range(4)], axis=1)
    w2m = np.stack([ch2_w[:, HID * j:HID * (j + 1)].T for j in range(4)],
                   axis=1)                              # [100h, 4, 100c]
    b2m = ch2_b[:, None]

    OWg = out_w * ln_h_g[None, :]
    owtm = np.zeros((101, HID), dtype=np.float32)
    owtm[0:100, :] = OWg.T
    owtm[100, :] = OWg.sum(axis=1)
    obm = (out_b + out_w @ ln_h_b)[:, None]

    pkm = np.zeros((SLOT, 4), dtype=np.float32)
    for b in range(4):
        pkm[30 * b:30 * b + 30, b] = 1.0 / K

    gtbm = np.zeros((SLOT, 2 * HID), dtype=np.float32)
    gtbm[:, :HID] = ln_t_g[None, :]
    gtbm[:, HID:] = ln_t_b[None, :]

    base = {
        "wht": wht.astype(BF16),
        "t1": t1m.astype(BF16),
        "t2": t2m.astype(BF16),
        "w1": w1m.astype(BF16),
        "w2": w2m.astype(BF16),
        "owt": owtm.astype(np.float32),
        "idb": np.eye(SLOT, dtype=np.float32).astype(BF16),
        "idf": np.eye(HID, dtype=np.float32),
        "b1": b1m.astype(np.float32),
        "b2": b2m.astype(np.float32),
        "ob": obm.astype(np.float32),
        "t1b": t1bv,
        "pk": pkm,
        "gtb": gtbm,
    }

    in_maps = []
    for c in range(NCORES):
        d = dense[c].reshape(NG, 4, 4, K, 106)       # [g, t, u, k, c]
        # pre-transposed: [g, feature, t, u*k] so the head matmul's lhsT
        # (GT) comes straight from DMA with no PE transpose
        d = np.ascontiguousarray(d.transpose(0, 4, 1, 2, 3))  # [g, c, t, u, k]
        d = d.reshape(NG, 106, 4 * SLOT)
        m = dict(base)
        m["xin"] = d
        in_maps.append(m)
    return in_maps, NG, NPC, NPCP, lnt_identity, N


def kernel(**inputs):
    global LAST_RESULT
    from concourse.bass_utils import run_bass_kernel_spmd

    in_maps, NG, NPC, NPCP, lnt_identity, N = _host_prepare(inputs)

    key = (NG, lnt_identity)
    if key not in _CACHE:
        _CACHE[key] = _build_nc(NG, lnt_identity)
    nc = _CACHE[key]

    res = run_bass_kernel_spmd(nc, in_maps, core_ids=list(range(NCORES)))
    LAST_RESULT = res

    parts = []
    remaining = N
    for c in range(NCORES):
        take = min(NPC, remaining)
        parts.append(res.results[c]["y2"].T[:take])
        remaining -= take
    out = np.ascontiguousarray(np.concatenate(parts, axis=0)).astype(np.float32)
    return out

